# revision 2
# baseline (speedup 1.0000x reference)
"""Causal self-attention Trainium2 kernel (8 NeuronCores, SPMD) — v2.

Sharding: 8 cores = 4 batches x 2 head-groups. Each core computes, for its
(batch b, head-group g): Q/K/V projections restricted to g's 8 heads
(column-parallel), causal attention for those heads, and the partial output
projection ctx_g @ Wo[g rows] (row-parallel). Host sums the two partials per
batch and adds the bias terms (bv @ Wo + bo).

v2 changes vs v1:
- Normalizer: no per-k-tile ones-matmuls (was ~68us of PE time). Instead the
  exp tiles are accumulated on DVE/GpSimd into acc[P,512] and ONE matmul
  ones[P,128]^T @ acc reduces over partitions AND broadcasts the row-sums to
  all 128 partitions in a single shot (no DRAM bounce, no 1-partition DVE
  reciprocal). Reciprocal runs on all 128 lanes.
- Diagonal trim: score/PV matmuls on diagonal k-tiles only cover the columns
  q >= k-tile start (N = 512-j*128 instead of 512).
- Output projection (P4) for q-block qb is interleaved at k-tile granularity
  into the attention loop of qb+1, so the PE keeps running when the scalar
  engine's exp stream is the local bottleneck.
- DMA head fix: first weight strips are issued before the xT chunks and
  split across both HWDGE rings, so the first matmul starts at ~2us instead
  of ~38us.
- Output is written in bf16 (host upcasts and sums the two group partials).
"""

import sys

sys.path.insert(0, "/opt/trn_rl_repo")

from collections import deque
from contextlib import ExitStack

import numpy as np

import concourse.bass as bass
import concourse.tile as tile
from concourse import mybir
from concourse.bass_utils import run_bass_kernel_spmd

BF16 = mybir.dt.bfloat16
F32 = mybir.dt.float32
NP_BF16 = mybir.dt.np(BF16)

# Problem constants (hardcoded per contract).
B = 4          # batch
S = 2048       # sequence length
DM = 2048      # d_model
H = 16         # total heads
HD = 128       # head dim
G = 2          # head groups (tensor parallel degree)
NHL = H // G   # local heads per core
DHL = NHL * HD # local head dims
NCORES = 8
P = 128        # partitions
FD = 512       # matmul moving free dim (one PSUM bank of f32)
NKC = DM // P  # contraction chunks for projections
NST = S // P   # seq tiles (k tiles)
NQB = S // FD  # 512-wide q blocks
SCALE = 1.0 / float(np.sqrt(HD))
MASK_VAL = -1e30

_WAIT_EXEMPT = {
    "NoOp",
    "EventSemaphore",
    "UnconditionalBranch",
    "RegisterMove",
    "ISA",
    "TileRelease",
}


def _fix_sync_waits(nc, max_waits=1):
    """Hoist extra sync-waits onto single-wait NoOps on the issuing engine.

    Several walrus instruction encodings (PSEUDO_DMA_DIRECT2D, S3_LW, CTRL_NO,
    ...) have a single sync-wait slot and fail codegen with "Too many sync
    wait commands" when Tile attaches more. A NoOp on the same engine
    immediately before the instruction performs the extra wait at the
    sequencer, which is semantically identical.
    """
    f = nc.m.functions[0]
    fixed = 0

    def walk(blocks):
        nonlocal fixed
        for b in blocks:
            il = b.instructions
            i = 0
            while i < len(il):
                inst = il[i]
                si = getattr(inst, "sync_info", None)
                ow = list(si.on_wait) if si is not None and si.on_wait else []
                if inst.opcode not in _WAIT_EXEMPT and len(ow) > max_waits:
                    keep = ow[len(ow) - max_waits :]
                    extra = ow[: len(ow) - max_waits]
                    for j, w in enumerate(extra):
                        nop = mybir.InstNoOp(
                            name=f"{inst.name}_waitfix{j}",
                            engine=inst.engine,
                            ins=[],
                            outs=[],
                            bass_nofuse=True,
                            sync_info=mybir.SyncInfo(on_wait=[w], on_update=[]),
                        )
                        il.insert(i, nop)
                        i += 1
                    inst.sync_info = mybir.SyncInfo(
                        on_wait=keep,
                        on_update=list(si.on_update) if si.on_update else [],
                    )
                    fixed += 1
                i += 1
            walk(getattr(b, "blocks", []) or [])

    walk(f.blocks)
    return fixed


def build_nc(fix_waits=True):
    """Build the single-core Bass program (same program for all 8 cores)."""
    nc = bass.Bass()
    # Inputs are pre-arranged on the host so every DMA line is contiguous.
    xT_d = nc.dram_tensor("xT", [P, NKC, S], BF16, kind="ExternalInput")
    wq_d = nc.dram_tensor("wq", [NHL, P, NKC, P], BF16, kind="ExternalInput")
    wk_d = nc.dram_tensor("wk", [NHL, P, NKC, P], BF16, kind="ExternalInput")
    wv_d = nc.dram_tensor("wv", [P, NKC, DHL], BF16, kind="ExternalInput")
    wo_d = nc.dram_tensor("wo", [P, DHL // P, DM], BF16, kind="ExternalInput")
    bqk_d = nc.dram_tensor("bqk", [P, 2, NHL], F32, kind="ExternalInput")
    out_d = nc.dram_tensor("out", [S, DM], BF16, kind="ExternalOutput")

    with tile.TileContext(nc) as tc:
        # ------------------------- pools (left stack) ---------------------
        es_main = ExitStack()
        consts = es_main.enter_context(tc.tile_pool(name="consts", bufs=1))
        bqk_sb = consts.tile([P, 2, NHL], F32)
        ones_sb = consts.tile([P, P], BF16)
        umask = consts.tile([P, P], F32)

        qkv = es_main.enter_context(tc.tile_pool(name="qkv", bufs=1))
        QT = qkv.tile([P, NHL, S], BF16)
        KT = qkv.tile([P, NHL, S], BF16)

        es_x = ExitStack()
        xpool = es_x.enter_context(tc.tile_pool(name="xpool", bufs=1))
        xT = xpool.tile([P, NKC, S], BF16)

        # ------------------------- pools (right stack) --------------------
        # LIFO close order: strips (end P1) -> wv (end P1b) -> V (end).
        es_v = ExitStack()
        vpool = es_v.enter_context(tc.tile_pool(name="vpool", bufs=1, side="right"))
        V = vpool.tile([P, NST, DHL], BF16)

        es_wv = ExitStack()
        wvpool = es_wv.enter_context(
            tc.tile_pool(name="wvpool", bufs=1, side="right")
        )
        wv_sb = wvpool.tile([P, NKC, DHL], BF16)

        es_strip = ExitStack()
        spool = es_strip.enter_context(
            tc.tile_pool(name="spool", bufs=4, side="right")
        )

        # ------------------------- constants setup ------------------------
        nc.vector.memset(ones_sb[:, :], 1.0)
        # umask[k, q] = 0 if q >= k else MASK_VAL (transposed diagonal block).
        nc.gpsimd.memset(umask[:, :], 0.0)
        nc.gpsimd.affine_select(
            out=umask[:, :],
            in_=umask[:, :],
            compare_op=mybir.AluOpType.is_ge,
            fill=MASK_VAL,
            base=0,
            pattern=[[1, P]],
            channel_multiplier=-1,
        )

        # ------------------------- DMA issue (order = priority) -----------
        # sync ring: bqk, h0 strips, then x chunks (even) + h1 strips.
        # scalar ring: x chunks (odd).
        # gpsimd (SWDGE) ring: wv now, wo later (after xT's SBUF is freed).
        strips = {}  # (h, 'q'|'k', half) -> tile

        def load_strip(h):
            for kind, src in (("q", wq_d), ("k", wk_d)):
                for half in range(2):
                    t = spool.tile(
                        [P, NKC // 2, P], BF16, tag="strip", name=f"w{kind}{h}_{half}"
                    )
                    nc.sync.dma_start(
                        out=t[:, :, :],
                        in_=src[h, :, half * (NKC // 2) : (half + 1) * (NKC // 2), :],
                    )
                    strips[(h, kind, half)] = t

        nc.sync.dma_start(out=bqk_sb[:, :, :], in_=bqk_d[:, :, :])
        load_strip(0)
        for i in range(NKC):
            eng = nc.sync if i % 2 == 0 else nc.scalar
            eng.dma_start(out=xT[:, i : i + 1, :], in_=xT_d[:, i : i + 1, :])
        load_strip(1)
        for i in range(0, NKC, 4):
            nc.gpsimd.dma_start(
                out=wv_sb[:, i : i + 4, :], in_=wv_d[:, i : i + 4, :]
            )

        # ------------------------- P1: QT / KT projections ----------------
        es_pp = ExitStack()
        ppsum = es_pp.enter_context(tc.tile_pool(name="ppsum", bufs=8, space="PSUM"))
        for h in range(NHL):
            if 2 <= h + 1 < NHL:
                load_strip(h + 1)
            for kind in ("q", "k"):
                ps = [
                    ppsum.tile([P, FD], F32, tag="pp", bufs=8, name=f"pp{kind}{h}_{qb}")
                    for qb in range(NQB)
                ]
                for c in range(NKC):
                    w = strips[(h, kind, c // (NKC // 2))]
                    for qb in range(NQB):
                        nc.tensor.matmul(
                            ps[qb][:, :],
                            w[:, c % (NKC // 2), :],
                            xT[:, c, qb * FD : (qb + 1) * FD],
                            start=(c == 0),
                            stop=(c == NKC - 1),
                        )
                dst = QT if kind == "q" else KT
                bias = bqk_sb[:, 0 if kind == "q" else 1, h : h + 1]
                for qb in range(NQB):
                    nc.scalar.activation(
                        dst[:, h, qb * FD : (qb + 1) * FD],
                        ps[qb][:, :],
                        mybir.ActivationFunctionType.Identity,
                        bias=bias,
                    )
        es_strip.close()
        es_pp.close()

        # ------------------------- P1b: V = x @ Wv ------------------------
        es_vp = ExitStack()
        vpsum = es_vp.enter_context(tc.tile_pool(name="vpsum", bufs=4, space="PSUM"))
        for st in range(NST):
            ps = [
                vpsum.tile([P, FD], F32, tag="vp", bufs=4, name=f"vp{st}_{dc}")
                for dc in range(2)
            ]
            for c in range(NKC):
                for dc in range(2):
                    nc.tensor.matmul(
                        ps[dc][:, :],
                        xT[:, c, st * P : (st + 1) * P],
                        wv_sb[:, c, dc * FD : (dc + 1) * FD],
                        start=(c == 0),
                        stop=(c == NKC - 1),
                    )
            for dc in range(2):
                nc.vector.tensor_copy(V[:, st, dc * FD : (dc + 1) * FD], ps[dc][:, :])
        es_vp.close()
        es_wv.close()
        es_x.close()

        # ------------------------- attention + out-proj -------------------
        # wo goes into the SBUF freed by xT (left stack, after es_x.close()).
        es_attn = ExitStack()
        wopool = es_attn.enter_context(tc.tile_pool(name="wopool", bufs=1))
        wo_sb = wopool.tile([P, DHL // P, DM], BF16)
        for i in range(0, DHL // P, 4):
            nc.gpsimd.dma_start(
                out=wo_sb[:, i : i + 4, :], in_=wo_d[:, i : i + 4, :]
            )

        epool = es_attn.enter_context(tc.tile_pool(name="epool", bufs=6))
        apool = es_attn.enter_context(tc.tile_pool(name="apool", bufs=2))
        rpool = es_attn.enter_context(tc.tile_pool(name="rpool", bufs=2))
        cpool = es_attn.enter_context(tc.tile_pool(name="cpool", bufs=2))
        stpool = es_attn.enter_context(tc.tile_pool(name="stpool", bufs=2))
        sps = es_attn.enter_context(tc.tile_pool(name="sps", bufs=4, space="PSUM"))
        pvs = es_attn.enter_context(tc.tile_pool(name="pvs", bufs=2, space="PSUM"))
        p4s = es_attn.enter_context(tc.tile_pool(name="p4s", bufs=2, space="PSUM"))

        # Pending out-projection micro-ops, popped into attention kt slots.
        p4q = deque()

        def queue_p4(qb, ctx):
            for stl in range(4):
                st = qb * 4 + stl
                box = {}

                def alloc(box=box, st=st):
                    box["stage"] = stpool.tile(
                        [P, DM], BF16, tag="stage", name=f"stage{st}"
                    )

                p4q.append(alloc)
                for half in range(2):

                    def mk_ps(box=box, st=st, half=half):
                        box["ps"] = [
                            p4s.tile([P, FD], F32, tag="p4", bufs=2, name=f"o{st}_{half}_{m}")
                            for m in range(2)
                        ]

                    p4q.append(mk_ps)
                    for dc in range(DHL // P):

                        def mm(box=box, stl=stl, half=half, dc=dc, ctx=ctx):
                            for m in range(2):
                                nc.tensor.matmul(
                                    box["ps"][m][:, :],
                                    ctx[:, dc, stl * P : (stl + 1) * P],
                                    wo_sb[:, dc, (half * 2 + m) * FD : (half * 2 + m + 1) * FD],
                                    start=(dc == 0),
                                    stop=(dc == DHL // P - 1),
                                )

                        p4q.append(mm)

                    def evict(box=box, half=half):
                        for m in range(2):
                            nc.scalar.copy(
                                box["stage"][:, (half * 2 + m) * FD : (half * 2 + m + 1) * FD],
                                box["ps"][m][:, :],
                            )

                    p4q.append(evict)

                def store(box=box, st=st):
                    nc.sync.dma_start(
                        out=out_d[st * P : (st + 1) * P, :], in_=box["stage"][:, :]
                    )

                p4q.append(store)

        def pop_p4(n):
            for _ in range(min(n, len(p4q))):
                p4q.popleft()()

        for qb in range(NQB):
            kmax = 4 * (qb + 1)
            ctx = cpool.tile([P, NHL, FD], BF16, tag="ctx", name=f"ctx{qb}")
            slots_left = NHL * kmax
            for h in range(NHL):
                acc = apool.tile([P, FD], BF16, tag="acc", name=f"acc{h}_{qb}")
                pv = pvs.tile([P, FD], F32, tag="pv", bufs=2, name=f"pv{h}_{qb}")
                exp_t = {}
                lo_of = {}
                for kt in range(kmax):
                    j = kt - 4 * qb
                    lo = max(j, 0) * P
                    lo_of[kt] = lo
                    sp = sps.tile([P, FD], F32, tag="sps", bufs=4, name=f"s{h}_{qb}_{kt}")
                    nc.tensor.matmul(
                        sp[:, lo:FD],
                        KT[:, h, kt * P : (kt + 1) * P],
                        QT[:, h, qb * FD + lo : (qb + 1) * FD],
                        start=True,
                        stop=True,
                    )
                    if j >= 0:
                        nc.vector.tensor_add(
                            sp[:, lo : lo + P], sp[:, lo : lo + P], umask[:, :]
                        )
                    ex = epool.tile([P, FD], BF16, tag="exp", name=f"e{h}_{qb}_{kt}")
                    nc.scalar.activation(
                        ex[:, lo:FD],
                        sp[:, lo:FD],
                        mybir.ActivationFunctionType.Exp,
                        scale=SCALE,
                    )
                    # Row-sum accumulation: full tiles on DVE, narrow diagonal
                    # tiles on the otherwise-idle GpSimd.
                    if kt == 0:
                        nc.gpsimd.tensor_copy(acc[:, :], ex[:, :])
                    elif j >= 0:
                        nc.gpsimd.tensor_add(
                            acc[:, lo:FD], acc[:, lo:FD], ex[:, lo:FD]
                        )
                    else:
                        nc.vector.tensor_add(acc[:, :], acc[:, :], ex[:, :])
                    exp_t[kt] = ex
                    if kt > 0:
                        pkt = kt - 1
                        plo = lo_of[pkt]
                        nc.tensor.matmul(
                            pv[:, plo:FD],
                            V[:, pkt, h * P : (h + 1) * P],
                            exp_t[pkt][:, plo:FD],
                            start=(pkt == 0),
                            stop=False,
                        )
                    # Interleave pending out-projection work for qb-1.
                    if p4q:
                        pop_p4(-(-len(p4q) // slots_left))
                    slots_left -= 1
                plo = lo_of[kmax - 1]
                nc.tensor.matmul(
                    pv[:, plo:FD],
                    V[:, kmax - 1, h * P : (h + 1) * P],
                    exp_t[kmax - 1][:, plo:FD],
                    start=(kmax == 1),
                    stop=True,
                )
                # Normalizer: partition-reduce + broadcast in one matmul.
                bc = sps.tile([P, FD], F32, tag="sps", bufs=4, name=f"bc{h}_{qb}")
                nc.tensor.matmul(
                    bc[:, :], ones_sb[:, :], acc[:, :], start=True, stop=True
                )
                recip = rpool.tile([P, FD], F32, tag="recip", name=f"r{h}_{qb}")
                nc.vector.reciprocal(recip[:, :], bc[:, :])
                nc.vector.tensor_mul(ctx[:, h, :], pv[:, :], recip[:, :])
            queue_p4(qb, ctx)
        while p4q:
            pop_p4(len(p4q))
        es_attn.close()
        es_v.close()
        es_main.close()

    if fix_waits:
        _fix_sync_waits(nc)
    return nc


def shard_inputs(x, Wq, bq, Wk, bk, Wv, bv, Wo, bo):
    """Host-side sharding: returns per-core input maps (bf16 pre-arranged)."""
    xTs = []
    for b in range(B):
        xt = np.ascontiguousarray(np.asarray(x)[b].T).astype(NP_BF16)  # [dm, seq]
        xTs.append(np.ascontiguousarray(xt.reshape(NKC, P, S).transpose(1, 0, 2)))
    wqs, wks, wvs, wos, bqks = [], [], [], [], []
    for g in range(G):
        sl = slice(g * DHL, (g + 1) * DHL)
        wq_s = np.asarray(Wq)[:, sl].astype(NP_BF16)
        wk_s = np.asarray(Wk)[:, sl].astype(NP_BF16)
        wv_s = np.asarray(Wv)[:, sl].astype(NP_BF16)
        wo_s = np.asarray(Wo)[sl, :].astype(NP_BF16)
        wqs.append(
            np.ascontiguousarray(wq_s.reshape(NKC, P, NHL, P).transpose(2, 1, 0, 3))
        )
        wks.append(
            np.ascontiguousarray(wk_s.reshape(NKC, P, NHL, P).transpose(2, 1, 0, 3))
        )
        wvs.append(np.ascontiguousarray(wv_s.reshape(NKC, P, DHL).transpose(1, 0, 2)))
        wos.append(
            np.ascontiguousarray(wo_s.reshape(DHL // P, P, DM).transpose(1, 0, 2))
        )
        bqk = np.stack(
            [
                np.asarray(bq, np.float32)[sl].reshape(NHL, P),
                np.asarray(bk, np.float32)[sl].reshape(NHL, P),
            ]
        )  # [2, nhl, P]
        bqks.append(np.ascontiguousarray(bqk.transpose(2, 0, 1)))  # [P, 2, nhl]
    in_maps = []
    for c in range(B * G):
        b, g = divmod(c, G)
        in_maps.append(
            {
                "xT": xTs[b],
                "wq": wqs[g],
                "wk": wks[g],
                "wv": wvs[g],
                "wo": wos[g],
                "bqk": bqks[g],
            }
        )
    return in_maps


_CACHE = {}


def _get_nc():
    if "nc" not in _CACHE:
        _CACHE["nc"] = build_nc()
    return _CACHE["nc"]


def run(inputs, trace=False):
    """Run the SPMD kernel; returns (full_output, BassKernelResults)."""
    inputs = {k: np.asarray(v) for k, v in inputs.items()}
    nc = _get_nc()
    in_maps = shard_inputs(**inputs)
    res = run_bass_kernel_spmd(
        nc, in_maps, core_ids=list(range(NCORES)), trace=trace
    )
    Wo = np.asarray(inputs["Wo"], np.float32)
    const_row = (
        np.asarray(inputs["bv"], np.float32) @ Wo + np.asarray(inputs["bo"], np.float32)
    )
    out = np.empty((B, S, DM), np.float32)
    for b in range(B):
        out[b] = (
            res.results[G * b]["out"].astype(np.float32)
            + res.results[G * b + 1]["out"].astype(np.float32)
            + const_row
        )
    return out, res


def kernel(**inputs):
    out, _ = run(inputs, trace=False)
    return out


# revision 4
# speedup vs baseline: 1.2240x; 1.2240x over previous
"""Causal self-attention Trainium2 kernel (8 NeuronCores, SPMD) — v2.

Sharding: 8 cores = 4 batches x 2 head-groups. Each core computes, for its
(batch b, head-group g): Q/K/V projections restricted to g's 8 heads
(column-parallel), causal attention for those heads, and the partial output
projection ctx_g @ Wo[g rows] (row-parallel). Host sums the two partials per
batch and adds the bias terms (bv @ Wo + bo).

v2 changes vs v1:
- Normalizer: no per-k-tile ones-matmuls (was ~68us of PE time). Instead the
  exp tiles are accumulated on DVE/GpSimd into acc[P,512] and ONE matmul
  ones[P,128]^T @ acc reduces over partitions AND broadcasts the row-sums to
  all 128 partitions in a single shot (no DRAM bounce, no 1-partition DVE
  reciprocal). Reciprocal runs on all 128 lanes.
- Diagonal trim: score/PV matmuls on diagonal k-tiles only cover the columns
  q >= k-tile start (N = 512-j*128 instead of 512).
- Output projection (P4) for q-block qb is interleaved at k-tile granularity
  into the attention loop of qb+1, so the PE keeps running when the scalar
  engine's exp stream is the local bottleneck.
- DMA head fix: first weight strips are issued before the xT chunks and
  split across both HWDGE rings, so the first matmul starts at ~2us instead
  of ~38us.
- Output is written in bf16 (host upcasts and sums the two group partials).
"""

import sys

sys.path.insert(0, "/opt/trn_rl_repo")

from collections import deque
from contextlib import ExitStack

import numpy as np

import concourse.bass as bass
import concourse.tile as tile
from concourse import mybir
from concourse.bass_utils import run_bass_kernel_spmd

BF16 = mybir.dt.bfloat16
F32 = mybir.dt.float32
NP_BF16 = mybir.dt.np(BF16)

# Problem constants (hardcoded per contract).
B = 4          # batch
S = 2048       # sequence length
DM = 2048      # d_model
H = 16         # total heads
HD = 128       # head dim
G = 2          # head groups (tensor parallel degree)
NHL = H // G   # local heads per core
DHL = NHL * HD # local head dims
NCORES = 8
P = 128        # partitions
FD = 512       # matmul moving free dim (one PSUM bank of f32)
NKC = DM // P  # contraction chunks for projections
NST = S // P   # seq tiles (k tiles)
NQB = S // FD  # 512-wide q blocks
SCALE = 1.0 / float(np.sqrt(HD))
MASK_VAL = -1e30

_WAIT_EXEMPT = {
    "NoOp",
    "EventSemaphore",
    "UnconditionalBranch",
    "RegisterMove",
    "ISA",
    "TileRelease",
}


def _fix_sync_waits(nc, max_waits=1):
    """Hoist extra sync-waits onto single-wait NoOps on the issuing engine.

    Several walrus instruction encodings (PSEUDO_DMA_DIRECT2D, S3_LW, CTRL_NO,
    ...) have a single sync-wait slot and fail codegen with "Too many sync
    wait commands" when Tile attaches more. A NoOp on the same engine
    immediately before the instruction performs the extra wait at the
    sequencer, which is semantically identical.
    """
    f = nc.m.functions[0]
    fixed = 0

    def walk(blocks):
        nonlocal fixed
        for b in blocks:
            il = b.instructions
            i = 0
            while i < len(il):
                inst = il[i]
                si = getattr(inst, "sync_info", None)
                ow = list(si.on_wait) if si is not None and si.on_wait else []
                if inst.opcode not in _WAIT_EXEMPT and len(ow) > max_waits:
                    keep = ow[len(ow) - max_waits :]
                    extra = ow[: len(ow) - max_waits]
                    for j, w in enumerate(extra):
                        nop = mybir.InstNoOp(
                            name=f"{inst.name}_waitfix{j}",
                            engine=inst.engine,
                            ins=[],
                            outs=[],
                            bass_nofuse=True,
                            sync_info=mybir.SyncInfo(on_wait=[w], on_update=[]),
                        )
                        il.insert(i, nop)
                        i += 1
                    inst.sync_info = mybir.SyncInfo(
                        on_wait=keep,
                        on_update=list(si.on_update) if si.on_update else [],
                    )
                    fixed += 1
                i += 1
            walk(getattr(b, "blocks", []) or [])

    walk(f.blocks)
    return fixed


def build_nc(fix_waits=True):
    """Build the single-core Bass program (same program for all 8 cores)."""
    nc = bass.Bass()
    # Inputs are pre-arranged on the host so every DMA line is contiguous.
    xT_d = nc.dram_tensor("xT", [P, NKC, S], BF16, kind="ExternalInput")
    wq_d = nc.dram_tensor("wq", [NHL, P, NKC, P], BF16, kind="ExternalInput")
    wk_d = nc.dram_tensor("wk", [NHL, P, NKC, P], BF16, kind="ExternalInput")
    wv_d = nc.dram_tensor("wv", [P, NKC, DHL], BF16, kind="ExternalInput")
    wo_d = nc.dram_tensor("wo", [P, DHL // P, DM], BF16, kind="ExternalInput")
    bqk_d = nc.dram_tensor("bqk", [P, 2, NHL], F32, kind="ExternalInput")
    out_d = nc.dram_tensor("out", [S, DM], BF16, kind="ExternalOutput")

    with tile.TileContext(nc) as tc:
        # ------------------------- pools (left stack) ---------------------
        es_main = ExitStack()
        consts = es_main.enter_context(tc.tile_pool(name="consts", bufs=1))
        bqk_sb = consts.tile([P, 2, NHL], F32)
        ones_sb = consts.tile([P, P], BF16)
        umask = consts.tile([P, P], F32)

        qkv = es_main.enter_context(tc.tile_pool(name="qkv", bufs=1))
        QT = qkv.tile([P, NHL, S], BF16)
        KT = qkv.tile([P, NHL, S], BF16)

        es_x = ExitStack()
        xpool = es_x.enter_context(tc.tile_pool(name="xpool", bufs=1))
        xT = xpool.tile([P, NKC, S], BF16)

        # ------------------------- pools (right stack) --------------------
        # LIFO close order: strips (end P1) -> wv (end P1b) -> V (end).
        es_v = ExitStack()
        vpool = es_v.enter_context(tc.tile_pool(name="vpool", bufs=1, side="right"))
        V = vpool.tile([P, NST, DHL], BF16)

        es_wv = ExitStack()
        wvpool = es_wv.enter_context(
            tc.tile_pool(name="wvpool", bufs=1, side="right")
        )
        wv_sb = wvpool.tile([P, NKC, DHL], BF16)

        es_strip = ExitStack()
        spool = es_strip.enter_context(
            tc.tile_pool(name="spool", bufs=4, side="right")
        )

        # ------------------------- constants setup ------------------------
        nc.vector.memset(ones_sb[:, :], 1.0)
        # umask[k, q] = 0 if q >= k else MASK_VAL (transposed diagonal block).
        nc.gpsimd.memset(umask[:, :], 0.0)
        nc.gpsimd.affine_select(
            out=umask[:, :],
            in_=umask[:, :],
            compare_op=mybir.AluOpType.is_ge,
            fill=MASK_VAL,
            base=0,
            pattern=[[1, P]],
            channel_multiplier=-1,
        )

        # ------------------------- DMA issue (order = priority) -----------
        # The two HWDGE rings (sync, scalar) serialize per ring but run each
        # transfer at full SDMA speed — use them for the critical-path head
        # loads (h0 strips + x chunks) in consumption order. Bulk loads that
        # aren't needed for a while (strips h1+, wv, wo) go on the gpsimd
        # SWDGE ring, which runs concurrently.
        strips = {}  # (h, 'q'|'k', half) -> tile

        def load_strip(h, eng):
            for kind, src in (("q", wq_d), ("k", wk_d)):
                for half in range(2):
                    t = spool.tile(
                        [P, NKC // 2, P], BF16, tag="strip", name=f"w{kind}{h}_{half}"
                    )
                    eng.dma_start(
                        out=t[:, :, :],
                        in_=src[h, :, half * (NKC // 2) : (half + 1) * (NKC // 2), :],
                    )
                    strips[(h, kind, half)] = t

        nc.scalar.dma_start(out=bqk_sb[:, :, :], in_=bqk_d[:, :, :])
        load_strip(0, nc.sync)
        for i in range(NKC):
            eng = nc.sync if i % 2 == 0 else nc.scalar
            eng.dma_start(out=xT[:, i : i + 1, :], in_=xT_d[:, i : i + 1, :])
        load_strip(1, nc.gpsimd)
        for i in range(0, NKC, 4):
            nc.gpsimd.dma_start(
                out=wv_sb[:, i : i + 4, :], in_=wv_d[:, i : i + 4, :]
            )

        # ------------------------- P1: QT / KT projections ----------------
        es_pp = ExitStack()
        ppsum = es_pp.enter_context(tc.tile_pool(name="ppsum", bufs=8, space="PSUM"))
        for h in range(NHL):
            if 2 <= h + 1 < NHL:
                load_strip(h + 1, nc.gpsimd)
            for kind in ("q", "k"):
                ps = [
                    ppsum.tile([P, FD], F32, tag="pp", bufs=8, name=f"pp{kind}{h}_{qb}")
                    for qb in range(NQB)
                ]
                for c in range(NKC):
                    w = strips[(h, kind, c // (NKC // 2))]
                    for qb in range(NQB):
                        nc.tensor.matmul(
                            ps[qb][:, :],
                            w[:, c % (NKC // 2), :],
                            xT[:, c, qb * FD : (qb + 1) * FD],
                            start=(c == 0),
                            stop=(c == NKC - 1),
                        )
                dst = QT if kind == "q" else KT
                bias = bqk_sb[:, 0 if kind == "q" else 1, h : h + 1]
                for qb in range(NQB):
                    nc.scalar.activation(
                        dst[:, h, qb * FD : (qb + 1) * FD],
                        ps[qb][:, :],
                        mybir.ActivationFunctionType.Identity,
                        bias=bias,
                    )
        es_strip.close()
        es_pp.close()

        # ------------------------- P1b: V = x @ Wv ------------------------
        es_vp = ExitStack()
        vpsum = es_vp.enter_context(tc.tile_pool(name="vpsum", bufs=4, space="PSUM"))
        for st in range(NST):
            ps = [
                vpsum.tile([P, FD], F32, tag="vp", bufs=4, name=f"vp{st}_{dc}")
                for dc in range(2)
            ]
            for c in range(NKC):
                for dc in range(2):
                    nc.tensor.matmul(
                        ps[dc][:, :],
                        xT[:, c, st * P : (st + 1) * P],
                        wv_sb[:, c, dc * FD : (dc + 1) * FD],
                        start=(c == 0),
                        stop=(c == NKC - 1),
                    )
            for dc in range(2):
                nc.vector.tensor_copy(V[:, st, dc * FD : (dc + 1) * FD], ps[dc][:, :])
        es_vp.close()
        es_wv.close()
        es_x.close()

        # ------------------------- attention + out-proj -------------------
        # wo goes into the SBUF freed by xT (left stack, after es_x.close()).
        es_attn = ExitStack()
        wopool = es_attn.enter_context(tc.tile_pool(name="wopool", bufs=1))
        wo_sb = wopool.tile([P, DHL // P, DM], BF16)
        for i in range(0, DHL // P, 4):
            nc.gpsimd.dma_start(
                out=wo_sb[:, i : i + 4, :], in_=wo_d[:, i : i + 4, :]
            )

        epool = es_attn.enter_context(tc.tile_pool(name="epool", bufs=6))
        apool = es_attn.enter_context(tc.tile_pool(name="apool", bufs=2))
        rpool = es_attn.enter_context(tc.tile_pool(name="rpool", bufs=2))
        cpool = es_attn.enter_context(tc.tile_pool(name="cpool", bufs=2))
        stpool = es_attn.enter_context(tc.tile_pool(name="stpool", bufs=2))
        sps = es_attn.enter_context(tc.tile_pool(name="sps", bufs=3, space="PSUM"))
        pvs = es_attn.enter_context(tc.tile_pool(name="pvs", bufs=3, space="PSUM"))
        p4s = es_attn.enter_context(tc.tile_pool(name="p4s", bufs=2, space="PSUM"))

        # Pending out-projection micro-ops, popped into attention kt slots.
        p4q = deque()

        def queue_p4(qb, ctx):
            for stl in range(4):
                st = qb * 4 + stl
                box = {}

                def alloc(box=box, st=st):
                    box["stage"] = stpool.tile(
                        [P, DM], BF16, tag="stage", name=f"stage{st}"
                    )

                p4q.append(alloc)
                for half in range(2):

                    def mk_ps(box=box, st=st, half=half):
                        box["ps"] = [
                            p4s.tile([P, FD], F32, tag="p4", bufs=2, name=f"o{st}_{half}_{m}")
                            for m in range(2)
                        ]

                    p4q.append(mk_ps)
                    for dc in range(DHL // P):

                        def mm(box=box, stl=stl, half=half, dc=dc, ctx=ctx):
                            for m in range(2):
                                nc.tensor.matmul(
                                    box["ps"][m][:, :],
                                    ctx[:, dc, stl * P : (stl + 1) * P],
                                    wo_sb[:, dc, (half * 2 + m) * FD : (half * 2 + m + 1) * FD],
                                    start=(dc == 0),
                                    stop=(dc == DHL // P - 1),
                                )

                        p4q.append(mm)

                    def evict(box=box, half=half):
                        for m in range(2):
                            nc.scalar.copy(
                                box["stage"][:, (half * 2 + m) * FD : (half * 2 + m + 1) * FD],
                                box["ps"][m][:, :],
                            )

                    p4q.append(evict)

                    def store(box=box, st=st, half=half):
                        eng = nc.sync if half == 0 else nc.scalar
                        eng.dma_start(
                            out=out_d[st * P : (st + 1) * P, half * 2 * FD : (half + 1) * 2 * FD],
                            in_=box["stage"][:, half * 2 * FD : (half + 1) * 2 * FD],
                        )

                    p4q.append(store)

        def pop_p4(n):
            for _ in range(min(n, len(p4q))):
                p4q.popleft()()

        for qb in range(NQB):
            kmax = 4 * (qb + 1)
            ctx = cpool.tile([P, NHL, FD], BF16, tag="ctx", name=f"ctx{qb}")
            slots_left = NHL * kmax
            for h in range(NHL):
                acc = apool.tile([P, FD], BF16, tag="acc", name=f"acc{h}_{qb}")
                pv = pvs.tile([P, FD], F32, tag="pv", bufs=3, name=f"pv{h}_{qb}")
                exp_t = {}
                lo_of = {}
                for kt in range(kmax):
                    j = kt - 4 * qb
                    lo = max(j, 0) * P
                    lo_of[kt] = lo
                    sp = sps.tile([P, FD], F32, tag="sps", bufs=3, name=f"s{h}_{qb}_{kt}")
                    nc.tensor.matmul(
                        sp[:, lo:FD],
                        KT[:, h, kt * P : (kt + 1) * P],
                        QT[:, h, qb * FD + lo : (qb + 1) * FD],
                        start=True,
                        stop=True,
                    )
                    if j >= 0:
                        nc.vector.tensor_add(
                            sp[:, lo : lo + P], sp[:, lo : lo + P], umask[:, :]
                        )
                    ex = epool.tile([P, FD], BF16, tag="exp", name=f"e{h}_{qb}_{kt}")
                    nc.scalar.activation(
                        ex[:, lo:FD],
                        sp[:, lo:FD],
                        mybir.ActivationFunctionType.Exp,
                        scale=SCALE,
                    )
                    # Row-sum accumulation on DVE (GpSimd's software tensor
                    # ops are ~5x slower and serialize the per-head chain).
                    if kt == 0:
                        nc.vector.tensor_copy(acc[:, :], ex[:, :])
                    else:
                        nc.vector.tensor_add(
                            acc[:, lo:FD], acc[:, lo:FD], ex[:, lo:FD]
                        )
                    exp_t[kt] = ex
                    if kt > 0:
                        pkt = kt - 1
                        plo = lo_of[pkt]
                        nc.tensor.matmul(
                            pv[:, plo:FD],
                            V[:, pkt, h * P : (h + 1) * P],
                            exp_t[pkt][:, plo:FD],
                            start=(pkt == 0),
                            stop=False,
                        )
                    # Interleave pending out-projection work for qb-1.
                    if p4q:
                        pop_p4(-(-len(p4q) // slots_left))
                    slots_left -= 1
                plo = lo_of[kmax - 1]
                nc.tensor.matmul(
                    pv[:, plo:FD],
                    V[:, kmax - 1, h * P : (h + 1) * P],
                    exp_t[kmax - 1][:, plo:FD],
                    start=(kmax == 1),
                    stop=True,
                )
                # Normalizer: partition-reduce + broadcast in one matmul.
                bc = sps.tile([P, FD], F32, tag="sps", bufs=3, name=f"bc{h}_{qb}")
                nc.tensor.matmul(
                    bc[:, :], ones_sb[:, :], acc[:, :], start=True, stop=True
                )
                recip = rpool.tile([P, FD], F32, tag="recip", name=f"r{h}_{qb}")
                nc.vector.reciprocal_approx_fast(out=recip[:, :], in_=bc[:, :])
                nc.vector.tensor_mul(ctx[:, h, :], pv[:, :], recip[:, :])
            queue_p4(qb, ctx)
        while p4q:
            pop_p4(len(p4q))
        es_attn.close()
        es_v.close()
        es_main.close()

    # Populate .instr bytes for the custom-DVE InstISA (reciprocal_approx) —
    # raw Bass skips this Bacc pass and the NEFF compiler rejects the empty
    # encoding with "ISA wrong length".
    mybir.codegen_inst_isa_subclasses(nc)
    if fix_waits:
        _fix_sync_waits(nc)
    return nc


def shard_inputs(x, Wq, bq, Wk, bk, Wv, bv, Wo, bo):
    """Host-side sharding: returns per-core input maps (bf16 pre-arranged)."""
    xTs = []
    for b in range(B):
        xt = np.ascontiguousarray(np.asarray(x)[b].T).astype(NP_BF16)  # [dm, seq]
        xTs.append(np.ascontiguousarray(xt.reshape(NKC, P, S).transpose(1, 0, 2)))
    wqs, wks, wvs, wos, bqks = [], [], [], [], []
    for g in range(G):
        sl = slice(g * DHL, (g + 1) * DHL)
        wq_s = np.asarray(Wq)[:, sl].astype(NP_BF16)
        wk_s = np.asarray(Wk)[:, sl].astype(NP_BF16)
        wv_s = np.asarray(Wv)[:, sl].astype(NP_BF16)
        wo_s = np.asarray(Wo)[sl, :].astype(NP_BF16)
        wqs.append(
            np.ascontiguousarray(wq_s.reshape(NKC, P, NHL, P).transpose(2, 1, 0, 3))
        )
        wks.append(
            np.ascontiguousarray(wk_s.reshape(NKC, P, NHL, P).transpose(2, 1, 0, 3))
        )
        wvs.append(np.ascontiguousarray(wv_s.reshape(NKC, P, DHL).transpose(1, 0, 2)))
        wos.append(
            np.ascontiguousarray(wo_s.reshape(DHL // P, P, DM).transpose(1, 0, 2))
        )
        bqk = np.stack(
            [
                np.asarray(bq, np.float32)[sl].reshape(NHL, P),
                np.asarray(bk, np.float32)[sl].reshape(NHL, P),
            ]
        )  # [2, nhl, P]
        bqks.append(np.ascontiguousarray(bqk.transpose(2, 0, 1)))  # [P, 2, nhl]
    in_maps = []
    for c in range(B * G):
        b, g = divmod(c, G)
        in_maps.append(
            {
                "xT": xTs[b],
                "wq": wqs[g],
                "wk": wks[g],
                "wv": wvs[g],
                "wo": wos[g],
                "bqk": bqks[g],
            }
        )
    return in_maps


_CACHE = {}


def _get_nc():
    if "nc" not in _CACHE:
        _CACHE["nc"] = build_nc()
    return _CACHE["nc"]


def run(inputs, trace=False):
    """Run the SPMD kernel; returns (full_output, BassKernelResults)."""
    inputs = {k: np.asarray(v) for k, v in inputs.items()}
    nc = _get_nc()
    in_maps = shard_inputs(**inputs)
    res = run_bass_kernel_spmd(
        nc, in_maps, core_ids=list(range(NCORES)), trace=trace
    )
    Wo = np.asarray(inputs["Wo"], np.float32)
    const_row = (
        np.asarray(inputs["bv"], np.float32) @ Wo + np.asarray(inputs["bo"], np.float32)
    )
    out = np.empty((B, S, DM), np.float32)
    for b in range(B):
        out[b] = (
            res.results[G * b]["out"].astype(np.float32)
            + res.results[G * b + 1]["out"].astype(np.float32)
            + const_row
        )
    return out, res


def kernel(**inputs):
    out, _ = run(inputs, trace=False)
    return out


# revision 5
# speedup vs baseline: 1.2474x; 1.0191x over previous
"""Causal self-attention Trainium2 kernel (8 NeuronCores, SPMD) — v2.

Sharding: 8 cores = 4 batches x 2 head-groups. Each core computes, for its
(batch b, head-group g): Q/K/V projections restricted to g's 8 heads
(column-parallel), causal attention for those heads, and the partial output
projection ctx_g @ Wo[g rows] (row-parallel). Host sums the two partials per
batch and adds the bias terms (bv @ Wo + bo).

v2 changes vs v1:
- Normalizer: no per-k-tile ones-matmuls (was ~68us of PE time). Instead the
  exp tiles are accumulated on DVE/GpSimd into acc[P,512] and ONE matmul
  ones[P,128]^T @ acc reduces over partitions AND broadcasts the row-sums to
  all 128 partitions in a single shot (no DRAM bounce, no 1-partition DVE
  reciprocal). Reciprocal runs on all 128 lanes.
- Diagonal trim: score/PV matmuls on diagonal k-tiles only cover the columns
  q >= k-tile start (N = 512-j*128 instead of 512).
- Output projection (P4) for q-block qb is interleaved at k-tile granularity
  into the attention loop of qb+1, so the PE keeps running when the scalar
  engine's exp stream is the local bottleneck.
- DMA head fix: first weight strips are issued before the xT chunks and
  split across both HWDGE rings, so the first matmul starts at ~2us instead
  of ~38us.
- Output is written in bf16 (host upcasts and sums the two group partials).
"""

import sys

sys.path.insert(0, "/opt/trn_rl_repo")

from collections import deque
from contextlib import ExitStack

import numpy as np

import concourse.bass as bass
import concourse.tile as tile
from concourse import mybir
from concourse.bass_utils import run_bass_kernel_spmd

BF16 = mybir.dt.bfloat16
F32 = mybir.dt.float32
NP_BF16 = mybir.dt.np(BF16)

# Problem constants (hardcoded per contract).
B = 4          # batch
S = 2048       # sequence length
DM = 2048      # d_model
H = 16         # total heads
HD = 128       # head dim
G = 2          # head groups (tensor parallel degree)
NHL = H // G   # local heads per core
DHL = NHL * HD # local head dims
NCORES = 8
P = 128        # partitions
FD = 512       # matmul moving free dim (one PSUM bank of f32)
NKC = DM // P  # contraction chunks for projections
NST = S // P   # seq tiles (k tiles)
NQB = S // FD  # 512-wide q blocks
SCALE = 1.0 / float(np.sqrt(HD))
MASK_VAL = -1e30

_WAIT_EXEMPT = {
    "NoOp",
    "EventSemaphore",
    "UnconditionalBranch",
    "RegisterMove",
    "ISA",
    "TileRelease",
}


def _fix_sync_waits(nc, max_waits=1):
    """Hoist extra sync-waits onto single-wait NoOps on the issuing engine.

    Several walrus instruction encodings (PSEUDO_DMA_DIRECT2D, S3_LW, CTRL_NO,
    ...) have a single sync-wait slot and fail codegen with "Too many sync
    wait commands" when Tile attaches more. A NoOp on the same engine
    immediately before the instruction performs the extra wait at the
    sequencer, which is semantically identical.
    """
    f = nc.m.functions[0]
    fixed = 0

    def walk(blocks):
        nonlocal fixed
        for b in blocks:
            il = b.instructions
            i = 0
            while i < len(il):
                inst = il[i]
                si = getattr(inst, "sync_info", None)
                ow = list(si.on_wait) if si is not None and si.on_wait else []
                if inst.opcode not in _WAIT_EXEMPT and len(ow) > max_waits:
                    keep = ow[len(ow) - max_waits :]
                    extra = ow[: len(ow) - max_waits]
                    for j, w in enumerate(extra):
                        nop = mybir.InstNoOp(
                            name=f"{inst.name}_waitfix{j}",
                            engine=inst.engine,
                            ins=[],
                            outs=[],
                            bass_nofuse=True,
                            sync_info=mybir.SyncInfo(on_wait=[w], on_update=[]),
                        )
                        il.insert(i, nop)
                        i += 1
                    inst.sync_info = mybir.SyncInfo(
                        on_wait=keep,
                        on_update=list(si.on_update) if si.on_update else [],
                    )
                    fixed += 1
                i += 1
            walk(getattr(b, "blocks", []) or [])

    walk(f.blocks)
    return fixed


def build_nc(fix_waits=True):
    """Build the single-core Bass program (same program for all 8 cores)."""
    nc = bass.Bass()
    # Inputs are pre-arranged on the host so every DMA line is contiguous.
    xT_d = nc.dram_tensor("xT", [P, NKC, S], BF16, kind="ExternalInput")
    wq_d = nc.dram_tensor("wq", [NHL, P, NKC, P], BF16, kind="ExternalInput")
    wk_d = nc.dram_tensor("wk", [NHL, P, NKC, P], BF16, kind="ExternalInput")
    wv_d = nc.dram_tensor("wv", [P, NKC, DHL], BF16, kind="ExternalInput")
    wo_d = nc.dram_tensor("wo", [P, DHL // P, DM], BF16, kind="ExternalInput")
    bqk_d = nc.dram_tensor("bqk", [P, 2, NHL], F32, kind="ExternalInput")
    out_d = nc.dram_tensor("out", [S, DM], BF16, kind="ExternalOutput")

    with tile.TileContext(nc) as tc:
        # ------------------------- pools (left stack) ---------------------
        es_main = ExitStack()
        consts = es_main.enter_context(tc.tile_pool(name="consts", bufs=1))
        bqk_sb = consts.tile([P, 2, NHL], F32)
        ones_sb = consts.tile([P, P], BF16)
        umask = consts.tile([P, P], F32)

        qkv = es_main.enter_context(tc.tile_pool(name="qkv", bufs=1))
        QT = qkv.tile([P, NHL, S], BF16)
        KT = qkv.tile([P, NHL, S], BF16)

        es_x = ExitStack()
        xpool = es_x.enter_context(tc.tile_pool(name="xpool", bufs=1))
        xT = xpool.tile([P, NKC, S], BF16)

        # ------------------------- pools (right stack) --------------------
        # LIFO close order: strips (end P1) -> wv (end P1b) -> V (end).
        es_v = ExitStack()
        vpool = es_v.enter_context(tc.tile_pool(name="vpool", bufs=1, side="right"))
        V = vpool.tile([P, NST, DHL], BF16)

        es_wv = ExitStack()
        wvpool = es_wv.enter_context(
            tc.tile_pool(name="wvpool", bufs=1, side="right")
        )
        wv_sb = wvpool.tile([P, NKC, DHL], BF16)

        es_strip = ExitStack()
        spool = es_strip.enter_context(
            tc.tile_pool(name="spool", bufs=6, side="right")
        )

        # ------------------------- constants setup ------------------------
        nc.vector.memset(ones_sb[:, :], 1.0)
        # umask[k, q] = 0 if q >= k else MASK_VAL (transposed diagonal block).
        nc.gpsimd.memset(umask[:, :], 0.0)
        nc.gpsimd.affine_select(
            out=umask[:, :],
            in_=umask[:, :],
            compare_op=mybir.AluOpType.is_ge,
            fill=MASK_VAL,
            base=0,
            pattern=[[1, P]],
            channel_multiplier=-1,
        )

        # ------------------------- DMA issue (order = priority) -----------
        # The SWDGE (gpsimd) ring spreads consecutive dma_starts across ~11
        # parallel queue rows, so splitting the head loads into many small
        # transfers issued in consumption order gives BOTH aggregate
        # bandwidth and in-order completion (an HWDGE ring serializes its
        # transfers at ~80 GB/s — far too slow for the 16 MB head stream).
        # x is split per (chunk, q-block) so each P1 matmul waits on exactly
        # its own 128 KB piece.
        strips = {}  # (h, 'q'|'k', half) -> tile

        def load_strip(h, eng):
            for kind, src in (("q", wq_d), ("k", wk_d)):
                for half in range(2):
                    t = spool.tile(
                        [P, NKC // 2, P], BF16, tag="strip", name=f"w{kind}{h}_{half}"
                    )
                    eng.dma_start(
                        out=t[:, :, :],
                        in_=src[h, :, half * (NKC // 2) : (half + 1) * (NKC // 2), :],
                    )
                    strips[(h, kind, half)] = t

        nc.sync.dma_start(out=bqk_sb[:, :, :], in_=bqk_d[:, :, :])
        for kind, src in (("q", wq_d), ("k", wk_d)):
            for half in range(2):
                t = spool.tile([P, NKC // 2, P], BF16, tag="strip", name=f"w{kind}0_{half}")
                nc.gpsimd.dma_start(
                    out=t[:, :, :],
                    in_=src[0, :, half * (NKC // 2) : (half + 1) * (NKC // 2), :],
                )
                strips[(0, kind, half)] = t
        for i in range(NKC):
            for qp in range(NQB):
                nc.gpsimd.dma_start(
                    out=xT[:, i, qp * FD : (qp + 1) * FD],
                    in_=xT_d[:, i, qp * FD : (qp + 1) * FD],
                )
        load_strip(1, nc.gpsimd)
        for i in range(0, NKC, 4):
            nc.gpsimd.dma_start(
                out=wv_sb[:, i : i + 4, :], in_=wv_d[:, i : i + 4, :]
            )

        # ------------------------- P1: QT / KT projections ----------------
        es_pp = ExitStack()
        ppsum = es_pp.enter_context(tc.tile_pool(name="ppsum", bufs=8, space="PSUM"))
        for h in range(NHL):
            if 2 <= h + 1 < NHL:
                load_strip(h + 1, nc.gpsimd)
            for kind in ("q", "k"):
                ps = [
                    ppsum.tile([P, FD], F32, tag="pp", bufs=8, name=f"pp{kind}{h}_{qb}")
                    for qb in range(NQB)
                ]
                for c in range(NKC):
                    w = strips[(h, kind, c // (NKC // 2))]
                    for qb in range(NQB):
                        nc.tensor.matmul(
                            ps[qb][:, :],
                            w[:, c % (NKC // 2), :],
                            xT[:, c, qb * FD : (qb + 1) * FD],
                            start=(c == 0),
                            stop=(c == NKC - 1),
                        )
                dst = QT if kind == "q" else KT
                bias = bqk_sb[:, 0 if kind == "q" else 1, h : h + 1]
                for qb in range(NQB):
                    nc.scalar.activation(
                        dst[:, h, qb * FD : (qb + 1) * FD],
                        ps[qb][:, :],
                        mybir.ActivationFunctionType.Identity,
                        bias=bias,
                    )
        es_strip.close()
        es_pp.close()

        # ------------------------- P1b: V = x @ Wv ------------------------
        es_vp = ExitStack()
        vpsum = es_vp.enter_context(tc.tile_pool(name="vpsum", bufs=4, space="PSUM"))
        for st in range(NST):
            ps = [
                vpsum.tile([P, FD], F32, tag="vp", bufs=4, name=f"vp{st}_{dc}")
                for dc in range(2)
            ]
            for c in range(NKC):
                for dc in range(2):
                    nc.tensor.matmul(
                        ps[dc][:, :],
                        xT[:, c, st * P : (st + 1) * P],
                        wv_sb[:, c, dc * FD : (dc + 1) * FD],
                        start=(c == 0),
                        stop=(c == NKC - 1),
                    )
            for dc in range(2):
                nc.vector.tensor_copy(V[:, st, dc * FD : (dc + 1) * FD], ps[dc][:, :])
        es_vp.close()
        es_wv.close()
        es_x.close()

        # ------------------------- attention + out-proj -------------------
        # wo goes into the SBUF freed by xT (left stack, after es_x.close()).
        es_attn = ExitStack()
        wopool = es_attn.enter_context(tc.tile_pool(name="wopool", bufs=1))
        wo_sb = wopool.tile([P, DHL // P, DM], BF16)
        for i in range(0, DHL // P, 4):
            nc.gpsimd.dma_start(
                out=wo_sb[:, i : i + 4, :], in_=wo_d[:, i : i + 4, :]
            )

        epool = es_attn.enter_context(tc.tile_pool(name="epool", bufs=6))
        apool = es_attn.enter_context(tc.tile_pool(name="apool", bufs=2))
        rpool = es_attn.enter_context(tc.tile_pool(name="rpool", bufs=2))
        cpool = es_attn.enter_context(tc.tile_pool(name="cpool", bufs=2))
        stpool = es_attn.enter_context(tc.tile_pool(name="stpool", bufs=2))
        sps = es_attn.enter_context(tc.tile_pool(name="sps", bufs=3, space="PSUM"))
        pvs = es_attn.enter_context(tc.tile_pool(name="pvs", bufs=3, space="PSUM"))
        p4s = es_attn.enter_context(tc.tile_pool(name="p4s", bufs=2, space="PSUM"))

        # Pending out-projection micro-ops, popped into attention kt slots.
        p4q = deque()

        def queue_p4(qb, ctx):
            for stl in range(4):
                st = qb * 4 + stl
                box = {}

                def alloc(box=box, st=st):
                    box["stage"] = stpool.tile(
                        [P, DM], BF16, tag="stage", name=f"stage{st}"
                    )

                p4q.append(alloc)
                for half in range(2):

                    def mk_ps(box=box, st=st, half=half):
                        box["ps"] = [
                            p4s.tile([P, FD], F32, tag="p4", bufs=2, name=f"o{st}_{half}_{m}")
                            for m in range(2)
                        ]

                    p4q.append(mk_ps)
                    for dc in range(DHL // P):

                        def mm(box=box, stl=stl, half=half, dc=dc, ctx=ctx):
                            for m in range(2):
                                nc.tensor.matmul(
                                    box["ps"][m][:, :],
                                    ctx[:, dc, stl * P : (stl + 1) * P],
                                    wo_sb[:, dc, (half * 2 + m) * FD : (half * 2 + m + 1) * FD],
                                    start=(dc == 0),
                                    stop=(dc == DHL // P - 1),
                                )

                        p4q.append(mm)

                    def evict_store(box=box, st=st, half=half):
                        for m in range(2):
                            mc = half * 2 + m
                            nc.scalar.copy(
                                box["stage"][:, mc * FD : (mc + 1) * FD],
                                box["ps"][m][:, :],
                            )
                            eng = nc.sync if m == 0 else nc.scalar
                            eng.dma_start(
                                out=out_d[st * P : (st + 1) * P, mc * FD : (mc + 1) * FD],
                                in_=box["stage"][:, mc * FD : (mc + 1) * FD],
                            )

                    p4q.append(evict_store)

        def pop_p4(n):
            for _ in range(min(n, len(p4q))):
                p4q.popleft()()

        for qb in range(NQB):
            kmax = 4 * (qb + 1)
            ctx = cpool.tile([P, NHL, FD], BF16, tag="ctx", name=f"ctx{qb}")
            slots_left = NHL * kmax
            for h in range(NHL):
                acc = apool.tile([P, FD], BF16, tag="acc", name=f"acc{h}_{qb}")
                pv = pvs.tile([P, FD], F32, tag="pv", bufs=3, name=f"pv{h}_{qb}")
                exp_t = {}
                lo_of = {}
                for kt in range(kmax):
                    j = kt - 4 * qb
                    lo = max(j, 0) * P
                    lo_of[kt] = lo
                    sp = sps.tile([P, FD], F32, tag="sps", bufs=3, name=f"s{h}_{qb}_{kt}")
                    nc.tensor.matmul(
                        sp[:, lo:FD],
                        KT[:, h, kt * P : (kt + 1) * P],
                        QT[:, h, qb * FD + lo : (qb + 1) * FD],
                        start=True,
                        stop=True,
                    )
                    if j >= 0:
                        nc.vector.tensor_add(
                            sp[:, lo : lo + P], sp[:, lo : lo + P], umask[:, :]
                        )
                    ex = epool.tile([P, FD], BF16, tag="exp", name=f"e{h}_{qb}_{kt}")
                    nc.scalar.activation(
                        ex[:, lo:FD],
                        sp[:, lo:FD],
                        mybir.ActivationFunctionType.Exp,
                        scale=SCALE,
                    )
                    # Row-sum accumulation on DVE (GpSimd's software tensor
                    # ops are ~5x slower and serialize the per-head chain).
                    if kt == 0:
                        nc.vector.tensor_copy(acc[:, :], ex[:, :])
                    else:
                        nc.vector.tensor_add(
                            acc[:, lo:FD], acc[:, lo:FD], ex[:, lo:FD]
                        )
                    exp_t[kt] = ex
                    if kt > 0:
                        pkt = kt - 1
                        plo = lo_of[pkt]
                        nc.tensor.matmul(
                            pv[:, plo:FD],
                            V[:, pkt, h * P : (h + 1) * P],
                            exp_t[pkt][:, plo:FD],
                            start=(pkt == 0),
                            stop=False,
                        )
                    # Interleave pending out-projection work for qb-1.
                    if p4q:
                        pop_p4(-(-len(p4q) // slots_left))
                    slots_left -= 1
                plo = lo_of[kmax - 1]
                nc.tensor.matmul(
                    pv[:, plo:FD],
                    V[:, kmax - 1, h * P : (h + 1) * P],
                    exp_t[kmax - 1][:, plo:FD],
                    start=(kmax == 1),
                    stop=True,
                )
                # Normalizer: partition-reduce + broadcast in one matmul.
                bc = sps.tile([P, FD], F32, tag="sps", bufs=3, name=f"bc{h}_{qb}")
                nc.tensor.matmul(
                    bc[:, :], ones_sb[:, :], acc[:, :], start=True, stop=True
                )
                recip = rpool.tile([P, FD], F32, tag="recip", name=f"r{h}_{qb}")
                nc.vector.reciprocal_approx_fast(out=recip[:, :], in_=bc[:, :])
                nc.vector.tensor_mul(ctx[:, h, :], pv[:, :], recip[:, :])
            queue_p4(qb, ctx)
        while p4q:
            pop_p4(len(p4q))
        es_attn.close()
        es_v.close()
        es_main.close()

    # Populate .instr bytes for the custom-DVE InstISA (reciprocal_approx) —
    # raw Bass skips this Bacc pass and the NEFF compiler rejects the empty
    # encoding with "ISA wrong length".
    mybir.codegen_inst_isa_subclasses(nc)
    if fix_waits:
        _fix_sync_waits(nc)
    return nc


def shard_inputs(x, Wq, bq, Wk, bk, Wv, bv, Wo, bo):
    """Host-side sharding: returns per-core input maps (bf16 pre-arranged)."""
    xTs = []
    for b in range(B):
        xt = np.ascontiguousarray(np.asarray(x)[b].T).astype(NP_BF16)  # [dm, seq]
        xTs.append(np.ascontiguousarray(xt.reshape(NKC, P, S).transpose(1, 0, 2)))
    wqs, wks, wvs, wos, bqks = [], [], [], [], []
    for g in range(G):
        sl = slice(g * DHL, (g + 1) * DHL)
        wq_s = np.asarray(Wq)[:, sl].astype(NP_BF16)
        wk_s = np.asarray(Wk)[:, sl].astype(NP_BF16)
        wv_s = np.asarray(Wv)[:, sl].astype(NP_BF16)
        wo_s = np.asarray(Wo)[sl, :].astype(NP_BF16)
        wqs.append(
            np.ascontiguousarray(wq_s.reshape(NKC, P, NHL, P).transpose(2, 1, 0, 3))
        )
        wks.append(
            np.ascontiguousarray(wk_s.reshape(NKC, P, NHL, P).transpose(2, 1, 0, 3))
        )
        wvs.append(np.ascontiguousarray(wv_s.reshape(NKC, P, DHL).transpose(1, 0, 2)))
        wos.append(
            np.ascontiguousarray(wo_s.reshape(DHL // P, P, DM).transpose(1, 0, 2))
        )
        bqk = np.stack(
            [
                np.asarray(bq, np.float32)[sl].reshape(NHL, P),
                np.asarray(bk, np.float32)[sl].reshape(NHL, P),
            ]
        )  # [2, nhl, P]
        bqks.append(np.ascontiguousarray(bqk.transpose(2, 0, 1)))  # [P, 2, nhl]
    in_maps = []
    for c in range(B * G):
        b, g = divmod(c, G)
        in_maps.append(
            {
                "xT": xTs[b],
                "wq": wqs[g],
                "wk": wks[g],
                "wv": wvs[g],
                "wo": wos[g],
                "bqk": bqks[g],
            }
        )
    return in_maps


_CACHE = {}


def _get_nc():
    if "nc" not in _CACHE:
        _CACHE["nc"] = build_nc()
    return _CACHE["nc"]


def run(inputs, trace=False):
    """Run the SPMD kernel; returns (full_output, BassKernelResults)."""
    inputs = {k: np.asarray(v) for k, v in inputs.items()}
    nc = _get_nc()
    in_maps = shard_inputs(**inputs)
    res = run_bass_kernel_spmd(
        nc, in_maps, core_ids=list(range(NCORES)), trace=trace
    )
    Wo = np.asarray(inputs["Wo"], np.float32)
    const_row = (
        np.asarray(inputs["bv"], np.float32) @ Wo + np.asarray(inputs["bo"], np.float32)
    )
    out = np.empty((B, S, DM), np.float32)
    for b in range(B):
        out[b] = (
            res.results[G * b]["out"].astype(np.float32)
            + res.results[G * b + 1]["out"].astype(np.float32)
            + const_row
        )
    return out, res


def kernel(**inputs):
    out, _ = run(inputs, trace=False)
    return out


# revision 6
# speedup vs baseline: 1.2914x; 1.0353x over previous
"""Causal self-attention Trainium2 kernel (8 NeuronCores, SPMD) — v2.

Sharding: 8 cores = 4 batches x 2 head-groups. Each core computes, for its
(batch b, head-group g): Q/K/V projections restricted to g's 8 heads
(column-parallel), causal attention for those heads, and the partial output
projection ctx_g @ Wo[g rows] (row-parallel). Host sums the two partials per
batch and adds the bias terms (bv @ Wo + bo).

v2 changes vs v1:
- Normalizer: no per-k-tile ones-matmuls (was ~68us of PE time). Instead the
  exp tiles are accumulated on DVE/GpSimd into acc[P,512] and ONE matmul
  ones[P,128]^T @ acc reduces over partitions AND broadcasts the row-sums to
  all 128 partitions in a single shot (no DRAM bounce, no 1-partition DVE
  reciprocal). Reciprocal runs on all 128 lanes.
- Diagonal trim: score/PV matmuls on diagonal k-tiles only cover the columns
  q >= k-tile start (N = 512-j*128 instead of 512).
- Output projection (P4) for q-block qb is interleaved at k-tile granularity
  into the attention loop of qb+1, so the PE keeps running when the scalar
  engine's exp stream is the local bottleneck.
- DMA head fix: first weight strips are issued before the xT chunks and
  split across both HWDGE rings, so the first matmul starts at ~2us instead
  of ~38us.
- Output is written in bf16 (host upcasts and sums the two group partials).
"""

import sys

sys.path.insert(0, "/opt/trn_rl_repo")

from collections import deque
from contextlib import ExitStack

import numpy as np

import concourse.bass as bass
import concourse.tile as tile
from concourse import mybir
from concourse.bass_utils import run_bass_kernel_spmd

BF16 = mybir.dt.bfloat16
F32 = mybir.dt.float32
NP_BF16 = mybir.dt.np(BF16)

# Problem constants (hardcoded per contract).
B = 4          # batch
S = 2048       # sequence length
DM = 2048      # d_model
H = 16         # total heads
HD = 128       # head dim
G = 2          # head groups (tensor parallel degree)
NHL = H // G   # local heads per core
DHL = NHL * HD # local head dims
NCORES = 8
P = 128        # partitions
FD = 512       # matmul moving free dim (one PSUM bank of f32)
NKC = DM // P  # contraction chunks for projections
NST = S // P   # seq tiles (k tiles)
NQB = S // FD  # 512-wide q blocks
SCALE = 1.0 / float(np.sqrt(HD))
MASK_VAL = -1e30

_WAIT_EXEMPT = {
    "NoOp",
    "EventSemaphore",
    "UnconditionalBranch",
    "RegisterMove",
    "ISA",
    "TileRelease",
}


def _fix_sync_waits(nc, max_waits=1):
    """Hoist extra sync-waits onto single-wait NoOps on the issuing engine.

    Several walrus instruction encodings (PSEUDO_DMA_DIRECT2D, S3_LW, CTRL_NO,
    ...) have a single sync-wait slot and fail codegen with "Too many sync
    wait commands" when Tile attaches more. A NoOp on the same engine
    immediately before the instruction performs the extra wait at the
    sequencer, which is semantically identical.
    """
    f = nc.m.functions[0]
    fixed = 0

    def walk(blocks):
        nonlocal fixed
        for b in blocks:
            il = b.instructions
            i = 0
            while i < len(il):
                inst = il[i]
                si = getattr(inst, "sync_info", None)
                ow = list(si.on_wait) if si is not None and si.on_wait else []
                if inst.opcode not in _WAIT_EXEMPT and len(ow) > max_waits:
                    keep = ow[len(ow) - max_waits :]
                    extra = ow[: len(ow) - max_waits]
                    for j, w in enumerate(extra):
                        nop = mybir.InstNoOp(
                            name=f"{inst.name}_waitfix{j}",
                            engine=inst.engine,
                            ins=[],
                            outs=[],
                            bass_nofuse=True,
                            sync_info=mybir.SyncInfo(on_wait=[w], on_update=[]),
                        )
                        il.insert(i, nop)
                        i += 1
                    inst.sync_info = mybir.SyncInfo(
                        on_wait=keep,
                        on_update=list(si.on_update) if si.on_update else [],
                    )
                    fixed += 1
                i += 1
            walk(getattr(b, "blocks", []) or [])

    walk(f.blocks)
    return fixed


def build_nc(fix_waits=True):
    """Build the single-core Bass program (same program for all 8 cores)."""
    nc = bass.Bass()
    # Inputs are pre-arranged on the host so every DMA line is contiguous.
    # wq/wk are half-strip-major so one [P, NKC//2, P] half is a contiguous
    # 2 KB line per partition (256 B lines are below SDMA line rate).
    xT_d = nc.dram_tensor("xT", [P, NKC, S], BF16, kind="ExternalInput")
    wq_d = nc.dram_tensor("wq", [NHL, 2, P, NKC // 2, P], BF16, kind="ExternalInput")
    wk_d = nc.dram_tensor("wk", [NHL, 2, P, NKC // 2, P], BF16, kind="ExternalInput")
    wv_d = nc.dram_tensor("wv", [P, NKC, DHL], BF16, kind="ExternalInput")
    wo_d = nc.dram_tensor("wo", [P, DHL // P, DM], BF16, kind="ExternalInput")
    bqk_d = nc.dram_tensor("bqk", [P, 2, NHL], F32, kind="ExternalInput")
    out_d = nc.dram_tensor("out", [S, DM], BF16, kind="ExternalOutput")

    with tile.TileContext(nc) as tc:
        # ------------------------- pools (left stack) ---------------------
        es_main = ExitStack()
        consts = es_main.enter_context(tc.tile_pool(name="consts", bufs=1))
        bqk_sb = consts.tile([P, 2, NHL], F32)
        ones_sb = consts.tile([P, P], BF16)
        umask = consts.tile([P, P], F32)

        qkv = es_main.enter_context(tc.tile_pool(name="qkv", bufs=1))
        QT = qkv.tile([P, NHL, S], BF16)
        KT = qkv.tile([P, NHL, S], BF16)

        es_x = ExitStack()
        xpool = es_x.enter_context(tc.tile_pool(name="xpool", bufs=1))
        xT = xpool.tile([P, NKC, S], BF16)

        # ------------------------- pools (right stack) --------------------
        # LIFO close order: strips (end P1) -> wv (end P1b) -> V (end).
        es_v = ExitStack()
        vpool = es_v.enter_context(tc.tile_pool(name="vpool", bufs=1, side="right"))
        V = vpool.tile([P, NST, DHL], BF16)

        es_wv = ExitStack()
        wvpool = es_wv.enter_context(
            tc.tile_pool(name="wvpool", bufs=1, side="right")
        )
        wv_sb = wvpool.tile([P, NKC, DHL], BF16)

        es_strip = ExitStack()
        spool = es_strip.enter_context(
            tc.tile_pool(name="spool", bufs=6, side="right")
        )

        # ------------------------- DMA issue (order = priority) -----------
        # The SWDGE (gpsimd) ring spreads consecutive dma_starts across ~11
        # parallel queue rows, giving aggregate bandwidth AND roughly
        # in-order completion (an HWDGE ring serializes its transfers at
        # ~80 GB/s — too slow for the 16 MB head stream). SWDGE issue costs
        # ~0.6us per dma_start, so x is split per (chunk, seq-half): 32
        # issues — fast enough to stay ahead of P1's ~0.85us/piece
        # consumption while still starting the first matmul at ~6us.
        strips = {}  # (h, 'q'|'k', half) -> tile

        def load_strip(h, eng):
            for kind, src in (("q", wq_d), ("k", wk_d)):
                for half in range(2):
                    t = spool.tile(
                        [P, NKC // 2, P], BF16, tag="strip", name=f"w{kind}{h}_{half}"
                    )
                    eng.dma_start(out=t[:, :, :], in_=src[h, half, :, :, :])
                    strips[(h, kind, half)] = t

        nc.sync.dma_start(out=bqk_sb[:, :, :], in_=bqk_d[:, :, :])
        load_strip(0, nc.gpsimd)
        for i in range(NKC):
            for hp in range(2):
                nc.gpsimd.dma_start(
                    out=xT[:, i, hp * (S // 2) : (hp + 1) * (S // 2)],
                    in_=xT_d[:, i, hp * (S // 2) : (hp + 1) * (S // 2)],
                )
        load_strip(1, nc.gpsimd)
        for i in range(0, NKC, 4):
            nc.gpsimd.dma_start(
                out=wv_sb[:, i : i + 4, :], in_=wv_d[:, i : i + 4, :]
            )

        # ------------------------- constants setup ------------------------
        nc.vector.memset(ones_sb[:, :], 1.0)
        # umask[k, q] = 0 if q >= k else MASK_VAL (transposed diagonal block).
        nc.gpsimd.memset(umask[:, :], 0.0)
        nc.gpsimd.affine_select(
            out=umask[:, :],
            in_=umask[:, :],
            compare_op=mybir.AluOpType.is_ge,
            fill=MASK_VAL,
            base=0,
            pattern=[[1, P]],
            channel_multiplier=-1,
        )

        # ------------------------- P1: QT / KT projections ----------------
        es_pp = ExitStack()
        ppsum = es_pp.enter_context(tc.tile_pool(name="ppsum", bufs=8, space="PSUM"))
        for h in range(NHL):
            if 2 <= h + 1 < NHL:
                load_strip(h + 1, nc.gpsimd)
            for kind in ("q", "k"):
                ps = [
                    ppsum.tile([P, FD], F32, tag="pp", bufs=8, name=f"pp{kind}{h}_{qb}")
                    for qb in range(NQB)
                ]
                for c in range(NKC):
                    w = strips[(h, kind, c // (NKC // 2))]
                    for qb in range(NQB):
                        nc.tensor.matmul(
                            ps[qb][:, :],
                            w[:, c % (NKC // 2), :],
                            xT[:, c, qb * FD : (qb + 1) * FD],
                            start=(c == 0),
                            stop=(c == NKC - 1),
                        )
                dst = QT if kind == "q" else KT
                bias = bqk_sb[:, 0 if kind == "q" else 1, h : h + 1]
                for qb in range(NQB):
                    nc.scalar.activation(
                        dst[:, h, qb * FD : (qb + 1) * FD],
                        ps[qb][:, :],
                        mybir.ActivationFunctionType.Identity,
                        bias=bias,
                    )
        es_strip.close()
        es_pp.close()

        # ------------------------- P1b: V = x @ Wv ------------------------
        es_vp = ExitStack()
        vpsum = es_vp.enter_context(tc.tile_pool(name="vpsum", bufs=4, space="PSUM"))
        for st in range(NST):
            ps = [
                vpsum.tile([P, FD], F32, tag="vp", bufs=4, name=f"vp{st}_{dc}")
                for dc in range(2)
            ]
            for c in range(NKC):
                for dc in range(2):
                    nc.tensor.matmul(
                        ps[dc][:, :],
                        xT[:, c, st * P : (st + 1) * P],
                        wv_sb[:, c, dc * FD : (dc + 1) * FD],
                        start=(c == 0),
                        stop=(c == NKC - 1),
                    )
            for dc in range(2):
                nc.vector.tensor_copy(V[:, st, dc * FD : (dc + 1) * FD], ps[dc][:, :])
        es_vp.close()
        es_wv.close()
        es_x.close()

        # ------------------------- attention + out-proj -------------------
        # wo goes into the SBUF freed by xT (left stack, after es_x.close()).
        es_attn = ExitStack()
        wopool = es_attn.enter_context(tc.tile_pool(name="wopool", bufs=1))
        wo_sb = wopool.tile([P, DHL // P, DM], BF16)
        for i in range(0, DHL // P, 4):
            nc.gpsimd.dma_start(
                out=wo_sb[:, i : i + 4, :], in_=wo_d[:, i : i + 4, :]
            )

        epool = es_attn.enter_context(tc.tile_pool(name="epool", bufs=6))
        apool = es_attn.enter_context(tc.tile_pool(name="apool", bufs=2))
        rpool = es_attn.enter_context(tc.tile_pool(name="rpool", bufs=2))
        cpool = es_attn.enter_context(tc.tile_pool(name="cpool", bufs=2))
        stpool = es_attn.enter_context(tc.tile_pool(name="stpool", bufs=2))
        sps = es_attn.enter_context(tc.tile_pool(name="sps", bufs=3, space="PSUM"))
        pvs = es_attn.enter_context(tc.tile_pool(name="pvs", bufs=2, space="PSUM"))
        bcs = es_attn.enter_context(tc.tile_pool(name="bcs", bufs=1, space="PSUM"))
        p4s = es_attn.enter_context(tc.tile_pool(name="p4s", bufs=2, space="PSUM"))
        pvb = es_attn.enter_context(tc.tile_pool(name="pvb", bufs=3))

        # Pending out-projection micro-ops, popped into attention kt slots.
        p4q = deque()

        def queue_p4(qb, ctx):
            for stl in range(4):
                st = qb * 4 + stl
                box = {}

                def alloc(box=box, st=st):
                    box["stage"] = stpool.tile(
                        [P, DM], BF16, tag="stage", name=f"stage{st}"
                    )

                p4q.append(alloc)
                for half in range(2):

                    def mk_ps(box=box, st=st, half=half):
                        box["ps"] = [
                            p4s.tile([P, FD], F32, tag="p4", bufs=2, name=f"o{st}_{half}_{m}")
                            for m in range(2)
                        ]

                    p4q.append(mk_ps)
                    for dc in range(DHL // P):

                        def mm(box=box, stl=stl, half=half, dc=dc, ctx=ctx):
                            for m in range(2):
                                nc.tensor.matmul(
                                    box["ps"][m][:, :],
                                    ctx[:, dc, stl * P : (stl + 1) * P],
                                    wo_sb[:, dc, (half * 2 + m) * FD : (half * 2 + m + 1) * FD],
                                    start=(dc == 0),
                                    stop=(dc == DHL // P - 1),
                                )

                        p4q.append(mm)

                    def evict_store(box=box, st=st, half=half):
                        for m in range(2):
                            mc = half * 2 + m
                            nc.scalar.copy(
                                box["stage"][:, mc * FD : (mc + 1) * FD],
                                box["ps"][m][:, :],
                            )
                            eng = (nc.sync, nc.scalar, nc.gpsimd)[(st * 4 + mc) % 3]
                            eng.dma_start(
                                out=out_d[st * P : (st + 1) * P, mc * FD : (mc + 1) * FD],
                                in_=box["stage"][:, mc * FD : (mc + 1) * FD],
                            )

                    p4q.append(evict_store)

        def pop_p4(n):
            for _ in range(min(n, len(p4q))):
                p4q.popleft()()

        for qb in range(NQB):
            kmax = 4 * (qb + 1)
            ctx = cpool.tile([P, NHL, FD], BF16, tag="ctx", name=f"ctx{qb}")
            slots_total = NHL * kmax
            slots_done = 0
            qlen0 = len(p4q)
            popped = 0
            for h in range(NHL):
                acc = apool.tile([P, FD], BF16, tag="acc", name=f"acc{h}_{qb}")
                pv = pvs.tile([P, FD], F32, tag="pv", bufs=2, name=f"pv{h}_{qb}")
                exp_t = {}
                lo_of = {}
                for kt in range(kmax):
                    j = kt - 4 * qb
                    lo = max(j, 0) * P
                    lo_of[kt] = lo
                    sp = sps.tile([P, FD], F32, tag="sps", bufs=3, name=f"s{h}_{qb}_{kt}")
                    nc.tensor.matmul(
                        sp[:, lo:FD],
                        KT[:, h, kt * P : (kt + 1) * P],
                        QT[:, h, qb * FD + lo : (qb + 1) * FD],
                        start=True,
                        stop=True,
                    )
                    if j >= 0:
                        nc.vector.tensor_add(
                            sp[:, lo : lo + P], sp[:, lo : lo + P], umask[:, :]
                        )
                    ex = epool.tile([P, FD], BF16, tag="exp", name=f"e{h}_{qb}_{kt}")
                    nc.scalar.activation(
                        ex[:, lo:FD],
                        sp[:, lo:FD],
                        mybir.ActivationFunctionType.Exp,
                        scale=SCALE,
                    )
                    # Row-sum accumulation on DVE (GpSimd's software tensor
                    # ops are ~5x slower and serialize the per-head chain).
                    if kt == 0:
                        nc.vector.tensor_copy(acc[:, :], ex[:, :])
                    else:
                        nc.vector.tensor_add(
                            acc[:, lo:FD], acc[:, lo:FD], ex[:, lo:FD]
                        )
                    exp_t[kt] = ex
                    if kt > 0:
                        pkt = kt - 1
                        plo = lo_of[pkt]
                        nc.tensor.matmul(
                            pv[:, plo:FD],
                            V[:, pkt, h * P : (h + 1) * P],
                            exp_t[pkt][:, plo:FD],
                            start=(pkt == 0),
                            stop=False,
                        )
                    # Interleave pending out-projection work for qb-1,
                    # paced evenly across this qb's kt slots.
                    slots_done += 1
                    target = qlen0 * slots_done // slots_total
                    while popped < target and p4q:
                        p4q.popleft()()
                        popped += 1
                plo = lo_of[kmax - 1]
                nc.tensor.matmul(
                    pv[:, plo:FD],
                    V[:, kmax - 1, h * P : (h + 1) * P],
                    exp_t[kmax - 1][:, plo:FD],
                    start=(kmax == 1),
                    stop=True,
                )
                # Free the pv PSUM bank promptly: copy to SBUF, normalize from
                # there (the recip chain then runs off the bank-critical path).
                pv_sb = pvb.tile([P, FD], F32, tag="pvb", name=f"pvb{h}_{qb}")
                nc.vector.tensor_copy(pv_sb[:, :], pv[:, :])
                # Normalizer: partition-reduce + broadcast in one matmul.
                bc = bcs.tile([P, FD], F32, tag="bc", bufs=1, name=f"bc{h}_{qb}")
                nc.tensor.matmul(
                    bc[:, :], ones_sb[:, :], acc[:, :], start=True, stop=True
                )
                recip = rpool.tile([P, FD], F32, tag="recip", name=f"r{h}_{qb}")
                nc.vector.reciprocal_approx_fast(out=recip[:, :], in_=bc[:, :])
                nc.vector.tensor_mul(ctx[:, h, :], pv_sb[:, :], recip[:, :])
            queue_p4(qb, ctx)
        while p4q:
            pop_p4(len(p4q))
        es_attn.close()
        es_v.close()
        es_main.close()

    # Populate .instr bytes for the custom-DVE InstISA (reciprocal_approx) —
    # raw Bass skips this Bacc pass and the NEFF compiler rejects the empty
    # encoding with "ISA wrong length".
    mybir.codegen_inst_isa_subclasses(nc)
    if fix_waits:
        _fix_sync_waits(nc)
    return nc


def shard_inputs(x, Wq, bq, Wk, bk, Wv, bv, Wo, bo):
    """Host-side sharding: returns per-core input maps (bf16 pre-arranged)."""
    xTs = []
    for b in range(B):
        xt = np.ascontiguousarray(np.asarray(x)[b].T).astype(NP_BF16)  # [dm, seq]
        xTs.append(np.ascontiguousarray(xt.reshape(NKC, P, S).transpose(1, 0, 2)))
    wqs, wks, wvs, wos, bqks = [], [], [], [], []
    for g in range(G):
        sl = slice(g * DHL, (g + 1) * DHL)
        wq_s = np.asarray(Wq)[:, sl].astype(NP_BF16)
        wk_s = np.asarray(Wk)[:, sl].astype(NP_BF16)
        wv_s = np.asarray(Wv)[:, sl].astype(NP_BF16)
        wo_s = np.asarray(Wo)[sl, :].astype(NP_BF16)
        wqs.append(
            np.ascontiguousarray(
                wq_s.reshape(2, NKC // 2, P, NHL, P).transpose(3, 0, 2, 1, 4)
            )
        )
        wks.append(
            np.ascontiguousarray(
                wk_s.reshape(2, NKC // 2, P, NHL, P).transpose(3, 0, 2, 1, 4)
            )
        )
        wvs.append(np.ascontiguousarray(wv_s.reshape(NKC, P, DHL).transpose(1, 0, 2)))
        wos.append(
            np.ascontiguousarray(wo_s.reshape(DHL // P, P, DM).transpose(1, 0, 2))
        )
        bqk = np.stack(
            [
                np.asarray(bq, np.float32)[sl].reshape(NHL, P),
                np.asarray(bk, np.float32)[sl].reshape(NHL, P),
            ]
        )  # [2, nhl, P]
        bqks.append(np.ascontiguousarray(bqk.transpose(2, 0, 1)))  # [P, 2, nhl]
    in_maps = []
    for c in range(B * G):
        b, g = divmod(c, G)
        in_maps.append(
            {
                "xT": xTs[b],
                "wq": wqs[g],
                "wk": wks[g],
                "wv": wvs[g],
                "wo": wos[g],
                "bqk": bqks[g],
            }
        )
    return in_maps


_CACHE = {}


def _get_nc():
    if "nc" not in _CACHE:
        _CACHE["nc"] = build_nc()
    return _CACHE["nc"]


def run(inputs, trace=False):
    """Run the SPMD kernel; returns (full_output, BassKernelResults)."""
    inputs = {k: np.asarray(v) for k, v in inputs.items()}
    nc = _get_nc()
    in_maps = shard_inputs(**inputs)
    res = run_bass_kernel_spmd(
        nc, in_maps, core_ids=list(range(NCORES)), trace=trace
    )
    Wo = np.asarray(inputs["Wo"], np.float32)
    const_row = (
        np.asarray(inputs["bv"], np.float32) @ Wo + np.asarray(inputs["bo"], np.float32)
    )
    out = np.empty((B, S, DM), np.float32)
    for b in range(B):
        out[b] = (
            res.results[G * b]["out"].astype(np.float32)
            + res.results[G * b + 1]["out"].astype(np.float32)
            + const_row
        )
    return out, res


def kernel(**inputs):
    out, _ = run(inputs, trace=False)
    return out
